# revision 32
# baseline (speedup 1.0000x reference)
"""EnsembleDeepSDF MoE-routing kernel for 8 Trainium2 NeuronCores.

Strategy: the harness calls kernel(**inputs) with the full inputs; we do all
routing on the host.  type_vec is sorted, so each expert owns a contiguous
segment of points.  We pick a per-core "phase shape" (tile counts per weight
slot, identical on every core so one SPMD program serves all 8 cores), pack
the 9 experts' segments into the 8*len(shape) single-expert slots, gather
each core's points (padding with point 0), and hand each core its own
pre-transposed/pre-scaled weight slots as inputs.  The device program is a
straight-line Tile kernel: per point-tile, 9 matmul layers with softplus
activations.

softplus: the compiler's ACT tables have no softplus, so we generate a
custom piecewise-cubic table (same binary format as the shipped sets,
reverse-engineered from exp's entries) that replaces `exp` with
softplus(x) = ln(1+e^x), and point the compiler at it via
BASS_ACT_ROOT_JSON_PATH.  One ACT op then does the whole activation
including the PSUM evacuation and the beta scale/bias fma (free on ACT).
A fallback "exact" mode (exp+ln+fused clamp/max on stock tables) is kept
behind KERNEL_SOFTPLUS=exact.

The torch Softplus(beta=100) is softplus(100*z)/100; we keep activations in
the H = softplus(100*z) domain and fold the 1/100 into the next layer's
weights host-side, so no extra scaling ops run on device.
"""

import json
import os as _os
import shutil
import tempfile

import numpy as np

T, D_IN, H, OUT, N_HID = 9, 67, 512, 1, 7
BETA = 100.0
N_CORES = 8
PT = 512          # points per tile (one PSUM bank of fp32)
P = 128           # partitions
KC = H // P       # 4 contraction chunks
MC = H // P       # 4 output-feature chunks
# point-tiles per pipeline step (psum tile = PAIR banks; PSUM holds
# 8//PAIR tiles).  PAIR=4 halves the per-op ACT pipeline-fill overhead
# and amortizes each DoubleRow LDWEIGHTS over 4 matmuls instead of 2, at
# the cost of a 2-deep (vs 4) psum rotation.
PAIR = int(_os.environ.get("KERNEL_PAIR", "4"))

# matmul dtype knob: "f32" (exact, 4 cyc/col), "f32r" (tf32-ish, 1 cyc/col),
# "bf16" (1 cyc/col, FWL halves LDWEIGHTS, halves DMA/SBUF bytes)
MM_MODE = _os.environ.get("KERNEL_MM_MODE", "bf16")
# "table" = custom softplus ACT table (1 op); "exact" = exp/ln chain
SOFTPLUS_MODE = _os.environ.get("KERNEL_SOFTPLUS", "table")
# fp8 DoubleRow for the 7 hidden-layer matmuls: weights are scaled into
# e4m3's normal range (x ALPHA) and activations carry a global x HS fold
# (baked into the softplus table); the ACT evacuation scale removes both.
# Measured numpy end-to-end rel err 4.6e-3 vs the 2e-2 gate.
FP8 = _os.environ.get("KERNEL_FP8", "1") != "0" and SOFTPLUS_MODE == "table" \
    and MM_MODE == "bf16"
HS = 0.125        # activation scale folded into the table
# fp8 weight scale.  ALPHA*HS == 1 so the hidden-layer PSUM arrives already
# in the BETA*z domain (ACT scale 1.0; the custom-DVE softplus needs no
# scale constant on its input).  max |Wh|*8 ~ 0.35, inside e4m3's normal
# range (min normal 2^-6).
ALPHA = 8.0
# custom-DVE piecewise softplus for half the hidden-layer evacuations.
# Numerically validated (rel err 5.7e-3) and compiles, but the runtime
# crashes executing the NEFF: the byte-36 row dispatch for custom-DVE ops
# appears to be baked into firmware (ant_dve_dispatch.hpp), so a row the
# firmware doesn't know is fatal.  Kept for documentation; off by default.
DVE_SP = _os.environ.get("KERNEL_DVE_SP", "0") != "0"
# fold wo into A7 on DVE + ones-matmul on PE for the output layer.
# Measured neutral (the stream is ACT-paced, so trimming PE's output-layer
# matmuls buys nothing) and it adds a bf16 rounding of the partial sums;
# kept behind a knob, off by default.
L8_DVE = _os.environ.get("KERNEL_L8_DVE", "0") != "0"

_nc_cache = {}
_last_results = None

# --------------------------------------------------------------------------
# Custom DVE op: 3-tangent piecewise-linear HS*softplus with bias add.
# out = max(0, HS/2*u + HS*(ln2+0.11), HS*u), u = psum + bias.  The +0.11
# lift centers the one-sided tangent error; measured end-to-end rel-err
# cost vs the exact table is ~3e-4.  Offloads 2 of 4 evacuation chunks per
# hidden layer from the saturated ACT engine (the softplus table runs only
# there) onto the mostly-idle DVE.
# --------------------------------------------------------------------------

_SP_PW3 = None
_SP_PW3_NAME = "SOFTPLUS_PW3_ANT"


def _get_sp_pw3():
    global _SP_PW3
    if _SP_PW3 is not None:
        return _SP_PW3
    from concourse.dve_spec import Spec, Src0, Src1, C0, C1, C2, Zero, maxx
    from concourse.dve_spec import _has_src1, lower
    from concourse.dve_uop import DveOpSpec
    import concourse.dve_ops as dops

    u = Src0 + Src1
    spec = Spec(body=maxx(maxx(C1 * u + C2, C0 * u), Zero))
    row = dops._CUSTOM_DVE_ROW_BASE + len(dops.OPS)
    shas = {}
    for ver in ("v3", "v4"):
        try:
            s = DveOpSpec(name=_SP_PW3_NAME, opcode=row,
                          uops=lower(spec, ver=ver), rd1_en=_has_src1(spec))
            shas[ver] = s.sha(ver)
        except Exception:
            pass
    _SP_PW3 = dops.DveOp(_SP_PW3_NAME, spec, subdim=False, uops_sha=shas)
    return _SP_PW3


def _register_sp_pw3():
    import concourse.dve_ops as dops
    op = _get_sp_pw3()
    if any(o.name == op.name for o in dops.OPS):
        return
    dops.OPS.append(op)
    dops.CUSTOM_DVE_SPECS[op.name] = op.spec
    dops._SUB_OPCODE_FOR_NAME[op.name] = (
        dops._CUSTOM_DVE_ROW_BASE + len(dops.OPS) - 1
    )


def _unregister_sp_pw3():
    import concourse.dve_ops as dops
    dops.OPS[:] = [o for o in dops.OPS if o.name != _SP_PW3_NAME]
    dops.CUSTOM_DVE_SPECS.pop(_SP_PW3_NAME, None)
    dops._SUB_OPCODE_FOR_NAME.pop(_SP_PW3_NAME, None)


# --------------------------------------------------------------------------
# Custom ACT table: replace `exp` with softplus in the shipped PWL sets.
# --------------------------------------------------------------------------

_ACT_SET = "natural_log_exp_and_others"
_act_table_dir = None


def _softplus64(x):
    x = np.asarray(x, dtype=np.float64)
    return np.log1p(np.exp(-np.abs(x))) + np.maximum(x, 0.0)


def _fit_cubic(a, b, hs):
    x0 = 0.5 * (a + b)
    k = np.arange(96)
    xs = x0 + 0.5 * (b - a) * np.cos(np.pi * (k + 0.5) / 96)
    c = np.polyfit(xs - x0, hs * _softplus64(xs), 3)
    return float(c[3]), float(c[2]), float(c[1]), float(c[0]), float(x0)


def _gen_act_tables(hs=1.0):
    """Build an act-root dir where `exp` computes hs*softplus. Returns the
    act_info.json path. The bucket entry layout ([d0,d1,d2,d3,x0,0,0,0],
    cubic in (x-x0)) and the per-exponent band structure are read from the
    shipped set so only coefficients and profile thresholds change."""
    global _act_table_dir
    if _act_table_dir is not None:
        return _act_table_dir
    from neuronxcc.driver.Job import Job
    from neuronxcc.driver.jobs.support.FindActInfo import findActInfoFile

    src_json = findActInfoFile(Job.getPackageDir(), "gen3")
    src = _os.path.dirname(src_json)
    out = _os.path.join(tempfile.mkdtemp(prefix="act_softplus_"), "tables")
    shutil.copytree(src, out)
    for f in _os.listdir(out):
        _os.chmod(_os.path.join(out, f), 0o644)

    d = json.load(open(f"{out}/{_ACT_SET}.json"))
    bkt = np.fromfile(f"{out}/{_ACT_SET}_bkt.bin", dtype=np.uint32)
    bkt = bkt.reshape(-1, 8).copy()
    fbkt = bkt.view(np.float32)
    e2b = {int(k): v for k, v in d["func_exp_to_bkt_start_idx"]["exp"].items()}
    prof = [p for p in d["profile_meta_data"] if p["func_name"] == "exp_400p"][0]

    def put(idx, d0, d1, d2, d3, x0):
        fbkt[idx, 0:5] = np.array([d0, d1, d2, d3, x0], dtype=np.float32)
        bkt[idx, 5:8] = 0

    nseg = {-1: 2, 0: 4, 1: 8, 2: 16, 3: 32}
    for e in range(-19, 4):
        n = nseg.get(e, 1)
        neg_base, pos_base = e2b[e]
        A = 2.0 ** e
        h = A / n
        for k in range(n):
            a, b = A + k * h, A + (k + 1) * h
            put(pos_base + k, *_fit_cubic(a, b, hs))
            put(neg_base + k, *_fit_cubic(-b, -a, hs))

    ln2 = float(np.log(2.0))
    put(prof["pos_small_signal_pwl_control"],
        hs * ln2, hs * 0.5, hs * 0.125, 0.0, 0.0)
    put(prof["neg_small_signal_pwl_control"],
        hs * ln2, hs * 0.5, hs * 0.125, 0.0, 0.0)
    put(prof["pos_large_signal_pwl_control"], 0.0, hs, 0.0, 0.0, 0.0)
    put(prof["neg_large_signal_pwl_control"], 0.0, 0.0, 0.0, 0.0, 0.0)
    prof["large_pos_signal_exp_threshold"] = 131   # |x| >= 16 -> linear/zero
    prof["large_pos_signal_mantissa_threshold"] = 0
    prof["large_neg_signal_exp_threshold"] = 131
    prof["large_neg_signal_mantissa_threshold"] = 0
    prof["fzero_result"] = int(np.float32(hs * ln2).view(np.uint32))
    prof["fninf_result"] = 0
    prof["fpinf_result"] = 2139095040

    bkt.tofile(f"{out}/{_ACT_SET}_bkt.bin")
    with open(f"{out}/{_ACT_SET}.json", "w") as f:
        json.dump(d, f)
    _act_table_dir = _os.path.join(out, "act_info.json")
    return _act_table_dir


# --------------------------------------------------------------------------
# Host-side planning: pack expert segments into 8 x len(shape) slots.
# --------------------------------------------------------------------------

def _try_pack(shape, counts):
    """Assign experts to single-expert slots. Slot (c, s) holds shape[s]*PT
    points. Returns {expert: [(core, s, amount), ...]} or None."""
    slots = []  # (capacity, core, s)
    for s, t in enumerate(shape):
        for c in range(N_CORES):
            slots.append([t * PT, c, s])
    experts = sorted(
        [e for e in range(T) if counts[e] > 0], key=lambda e: -counts[e]
    )
    asg = {}
    avail = sorted(slots)  # by capacity asc
    for e in experts:
        need = int(counts[e])
        # smallest single slot that fits
        one = next((sl for sl in avail if sl[0] >= need), None)
        if one is not None:
            asg[e] = [(one[1], one[2], need)]
            avail.remove(one)
            continue
        # greedily take largest slots
        take = []
        rem = need
        pool = sorted(avail, key=lambda sl: -sl[0])
        for sl in pool:
            if rem <= 0:
                break
            amt = min(rem, sl[0])
            take.append((sl[1], sl[2], amt))
            rem -= amt
            avail.remove(sl)
        if rem > 0:
            return None
        asg[e] = take
    return asg


def _plan(counts):
    cands = set()
    for t1 in range(1, 17):
        cands.add((t1,))
        for t2 in range(1, t1 + 1):
            cands.add((t1, t2))
            for t3 in range(1, t2 + 1):
                cands.add((t1, t2, t3))
    for shape in sorted(cands, key=lambda s: (sum(s), len(s))):
        asg = _try_pack(shape, counts)
        if asg is not None:
            return shape, asg
    raise RuntimeError("no feasible slot shape")


# --------------------------------------------------------------------------
# Device program
# --------------------------------------------------------------------------

def _build_nc(caps, mm_mode):
    import concourse.bass as bass
    import concourse.tile as tile
    import concourse.mybir as mybir
    from concourse import bacc

    f32 = mybir.dt.float32
    AF = mybir.ActivationFunctionType
    ALU = mybir.AluOpType
    if mm_mode == "bf16":
        wdt = mybir.dt.bfloat16   # weights/x/h (matmul operands)
        udt = f32                 # u stays f32; h is a separate bf16 tile
    elif mm_mode == "f32r":
        wdt = mybir.dt.float32r
        udt = mybir.dt.float32r   # u doubles as h (in-place max)
    else:
        wdt = f32
        udt = f32

    S = len(caps)
    NP = sum(caps)

    hdt = mybir.dt.float8e4 if FP8 else wdt   # hidden-layer matmul operands

    nc = bacc.Bacc("TRN2", target_bir_lowering=False)
    xT_in = nc.dram_tensor("xT", [D_IN, NP], wdt, kind="ExternalInput")
    w0t_in = nc.dram_tensor("w0t", [S, D_IN, H], wdt, kind="ExternalInput")
    wht_in = nc.dram_tensor("wht", [S, N_HID, P, KC, H], hdt, kind="ExternalInput")
    wot_in = nc.dram_tensor("wot", [S, P, KC], wdt, kind="ExternalInput")
    wov_in = nc.dram_tensor("wov", [S, P, KC], f32, kind="ExternalInput")
    b0v_in = nc.dram_tensor("b0v", [S, P, MC], f32, kind="ExternalInput")
    bhv_in = nc.dram_tensor("bhv", [S, P, N_HID, MC], f32, kind="ExternalInput")
    bov_in = nc.dram_tensor("bov", [S, 1], f32, kind="ExternalInput")
    out_d = nc.dram_tensor("out", [1, NP], f32, kind="ExternalOutput")

    # steps: (point_offset, (tile_sizes...), slot); each step's tiles go in
    # one PSUM tile (first tile bank-aligned at 512, total <= 1024)
    steps = []
    off = 0
    for s, cap in enumerate(caps):
        sizes = [PT] * (cap // PT)
        if cap % PT:
            sizes.append(cap % PT)
        i = 0
        while i < len(sizes):
            take = sizes[i:i + PAIR]
            steps.append((off, tuple(take), s))
            off += sum(take)
            i += PAIR

    NSTREAM = int(_os.environ.get("KERNEL_NSTREAM", "0")) or len(steps)
    NWARM = int(_os.environ.get("KERNEL_NWARM", "112"))

    with tile.TileContext(nc) as tc:
        with (
            tc.tile_pool(name="xin", bufs=len(steps)) as xin_pool,
            tc.tile_pool(name="wts", bufs=1) as wts_pool,
            tc.tile_pool(
                name="whp",
                bufs=(S * N_HID if mm_mode == "bf16"
                      else min(10 if SOFTPLUS_MODE == "table" else 7, S * N_HID)),
            ) as wh_pool,
            tc.tile_pool(name="uh", bufs=3 if mm_mode == "bf16" else 2 * NSTREAM) as uh_pool,
            tc.tile_pool(name="hb", bufs=2 * NSTREAM) as hb_pool,
            tc.tile_pool(name="ebuf", bufs=2) as e_pool,
            tc.tile_pool(name="outp", bufs=2) as out_pool,
            tc.tile_pool(name="ps", bufs=8 // PAIR, space="PSUM") as ps_pool,
        ):
            groups = [steps[i:i + NSTREAM] for i in range(0, len(steps), NSTREAM)]
            xT_sb = {}
            h_cur = {}

            # DMA trigger economics: each dma_start costs ~1.3us of the
            # issuing engine's sequencer, triggers on one queue serialize,
            # and one trigger's transfer runs on a single DMA engine
            # (~22.5 GB/s).  So the first-layer-critical loads are split
            # across BOTH hwdge queues (sync + scalar; scalar is free until
            # the first softplus at ~6us) with 2-way transfer splits, and
            # the long weight tail streams on the gpsimd software-DGE queue
            # in consumption order.
            w0_sb, wo_sb, b0_sb, bh_sb, bo_sb = [None] * S, [None] * S, [None] * S, [None] * S, [None] * S
            wov_sb = [None] * S
            wh_sb = [[None] * N_HID for _ in range(S)]

            def load_wh(s, l, splits):
                """splits: list of (engine, kc_lo, kc_hi) DMA pieces.  Layer
                matmuls consume kc chunks in order, and Tile's AP-range deps
                let kc0/1 matmuls start before kc2/3 land."""
                wh_t = wh_pool.tile([P, KC, H], hdt, name=f"wh_{s}_{l}", tag="wh")
                for eng, lo, hi in splits:
                    eng.dma_start(wh_t[:, lo:hi, :], wht_in[s, l, :, lo:hi, :])
                wh_sb[s][l] = wh_t

            def load_x(t0, szs, eng, nsplit=1):
                x_t = xin_pool.tile([D_IN, PAIR * PT], wdt,
                                    name=f"x_{t0}", tag="x")
                w = sum(szs)
                step = (w + nsplit - 1) // nsplit
                for o in range(0, w, step):
                    e = min(o + step, w)
                    eng.dma_start(x_t[:, o:e], xT_in[:, t0 + o:t0 + e])
                xT_sb[t0] = x_t

            def load_w0b0(s, eng):
                w0_t = wts_pool.tile([D_IN, H], wdt, name=f"w0_{s}")
                eng.dma_start(w0_t[:], w0t_in[s])
                w0_sb[s] = w0_t
                b0_t = wts_pool.tile([P, MC], f32, name=f"b0_{s}")
                eng.dma_start(b0_t[:], b0v_in[s])
                b0_sb[s] = b0_t

            def load_bh(s, eng):
                bh_t = wts_pool.tile([P, N_HID, MC], f32, name=f"bh_{s}")
                eng.dma_start(bh_t[:], bhv_in[s])
                bh_sb[s] = bh_t

            def load_wobo(s, eng):
                wo_t = wts_pool.tile([P, KC], wdt, name=f"wo_{s}")
                eng.dma_start(wo_t[:], wot_in[s])
                wo_sb[s] = wo_t
                wov_t = wts_pool.tile([P, KC], f32, name=f"wov_{s}")
                eng.dma_start(wov_t[:], wov_in[s])
                wov_sb[s] = wov_t
                bo_t = wts_pool.tile([1, 1], f32, name=f"bo_{s}")
                eng.dma_start(bo_t[:], bov_in[s:s + 1, 0:1])
                bo_sb[s] = bo_t

            # pre-warm the ACT table set during the initial DMA wait: a
            # dependency-free dummy op carries the one-time table load
            warm_t = wts_pool.tile([1, 1], f32, name="warm")
            nc.vector.memset(warm_t[:], 0.0)
            nc.scalar.activation(warm_t[:], warm_t[:], AF.Exp)
            # ones column for the output layer's cross-partition sum
            ones_t = wts_pool.tile([P, 1], wdt, name="ones")
            nc.vector.memset(ones_t[:], 1.0)

            # Small weight/bias tensors ride the sync HWDGE queue — its own
            # sequencer issues triggers (~1.2us each) in parallel with the
            # gpsimd queue's, so the first-layer critical set (w0+b0) lands
            # ~10us earlier than when queued behind the x pieces.  x and
            # the wh bulk stay on the gpsimd SWDGE queue (8 DMA engines, 8
            # transfers in flight, in-order completion ring), in
            # consumption order: per layer, alternating slots.
            slot_order = []
            for (_t0, _szs, s) in steps:
                if s not in slot_order:
                    slot_order.append(s)
            s0 = slot_order[0]

            for s in slot_order:
                load_w0b0(s, nc.sync)
                load_bh(s, nc.sync)

            for (t0, szs, _s) in steps[:2]:
                load_x(t0, szs, nc.gpsimd, nsplit=2)
            load_wh(s0, 0, [(nc.gpsimd, 0, 1), (nc.gpsimd, 1, 2),
                            (nc.gpsimd, 2, 3), (nc.gpsimd, 3, 4)])
            for (t0, szs, _s) in steps[2:4]:
                load_x(t0, szs, nc.gpsimd, nsplit=2)
            for (t0, szs, _s) in steps[4:]:
                load_x(t0, szs, nc.gpsimd)
            for s in slot_order:
                load_wobo(s, nc.sync)
            for l in range(N_HID):
                for s in slot_order:
                    if s == s0 and l == 0:
                        continue
                    load_wh(s, l, [(nc.gpsimd, 0, 2), (nc.gpsimd, 2, 4)])

            # HAM warm-up: dependency-free matmuls on a memset tile keep the
            # PE busy (and the clock gate at 8/8 = 2.4 GHz) through the
            # initial x/weight DMA wait.  An idle PE not only wastes that
            # window — it re-throttles to 1.2 GHz and takes >3.4us of busy
            # time to recover, so bridging the whole wait is worth it.
            wdum = wts_pool.tile([P, P], wdt, name="wdum")
            nc.vector.memset(wdum[:], 0.0)
            ps_warm = ps_pool.tile([P, PAIR * PT], f32, name="ps_warm", tag="ps")
            for _ in range(NWARM):
                nc.tensor.matmul(
                    ps_warm[:, 0:P], wdum[:], wdum[:], start=True, stop=True
                )

            def emit_mms(t0, szs, s, l):
                """Matmuls for layer l + PSUM evacuation into u (the evac ops
                are emitted here so they sit at the head of the DVE/ACT queues
                and free PSUM slots promptly)."""
                npts = sum(szs)
                # tile-local offsets; all but the last tile are 512 so
                # every tile stays bank-aligned in PSUM
                locs = [sum(szs[:i]) for i in range(len(szs))]
                h_prev = h_cur.get(t0)
                psums = []
                for mc in range(MC):
                    ps_t = ps_pool.tile(
                        [P, PAIR * PT], f32, name=f"ps_{t0}_{l}_{mc}", tag="ps"
                    )
                    psums.append(ps_t)
                    if l == 0:
                        for loc, sz in zip(locs, szs):
                            nc.tensor.matmul(
                                ps_t[:, loc:loc + sz],
                                w0_sb[s][:, mc * P:(mc + 1) * P],
                                xT_sb[t0][:, loc:loc + sz],
                                start=True, stop=True,
                            )
                    elif FP8:
                        # fp8 DoubleRow: each matmul contracts a kc PAIR
                        # (2x128 rows, 2 fp8 weights per PE cell)
                        for kp in range(KC // 2):
                            for loc, sz in zip(locs, szs):
                                nc.tensor.matmul(
                                    ps_t[:, loc:loc + sz],
                                    wh_sb[s][l - 1][
                                        :, 2 * kp:2 * kp + 2,
                                        mc * P:(mc + 1) * P],
                                    h_prev[:, 2 * kp:2 * kp + 2,
                                           loc:loc + sz],
                                    start=(kp == 0), stop=(kp == KC // 2 - 1),
                                    perf_mode=mybir.MatmulPerfMode.DoubleRow,
                                )
                    else:
                        for kc in range(KC):
                            for loc, sz in zip(locs, szs):
                                nc.tensor.matmul(
                                    ps_t[:, loc:loc + sz],
                                    wh_sb[s][l - 1][:, kc, mc * P:(mc + 1) * P],
                                    h_prev[:, kc, loc:loc + sz],
                                    start=(kc == 0), stop=(kc == KC - 1),
                                )
                # activation tiles feeding fp8 matmuls are fp8 themselves
                adt = (mybir.dt.float8e4 if FP8 and l < N_HID else wdt)
                if SOFTPLUS_MODE == "table" and mm_mode == "bf16":
                    u_t = hb_pool.tile([P, MC, PAIR * PT], adt,
                                       name=f"u_{t0}_{l}", tag="hb")
                else:
                    u_t = uh_pool.tile([P, MC, PAIR * PT], udt,
                                       name=f"u_{t0}_{l}", tag="uh")
                # table arg must be BETA*z.  fp8 layers: psum = A@W_devT =
                # (HS*BETA*h)@(ALPHA*Wh)T -> scale 1/(HS*ALPHA); l==0: psum =
                # x@W0T = z0-b0 -> scale BETA; bf16 hidden (non-fp8): weights
                # pre-scaled 1/BETA host-side -> scale BETA.
                act_scale = float(BETA) if l == 0 or not FP8 \
                    else float(1.0 / (HS * ALPHA))
                for mc in range(MC):
                    bias = (b0_sb[s][:, mc:mc + 1] if l == 0
                            else bh_sb[s][:, l - 1, mc:mc + 1])
                    if (SOFTPLUS_MODE == "table" and FP8 and mc < 2
                            and 1 <= l < N_HID and DVE_SP):
                        # ACT is the saturated engine (softplus table); move
                        # half of each fp8 hidden layer's evacuation to DVE
                        # with the piecewise-linear softplus.  Needs the
                        # pre-scaled PSUM (ALPHA*HS == 1), so l == 0 (and
                        # the exact A7 for the cancellation-sensitive output
                        # layer) stay on the ACT table.
                        nc.vector._custom_dve(
                            _get_sp_pw3(),
                            out=u_t[:, mc, 0:npts],
                            in0=psums[mc][:, 0:npts],
                            in1=bias,
                            s0=float(HS), s1=float(HS / 2),
                            imm2=float(HS * (np.log(2.0) + 0.11)),
                        )
                    elif SOFTPLUS_MODE == "table":
                        # hijacked Exp == softplus; one ACT op does the
                        # evacuation + beta fma + activation
                        nc.scalar.activation(
                            u_t[:, mc, 0:npts], psums[mc][:, 0:npts],
                            AF.Exp, bias=bias, scale=act_scale,
                        )
                    elif mc < 3:
                        # u = 100*y + 100*b; evac split DVE (mc 0-2) / ACT (3)
                        nc.vector.tensor_scalar(
                            u_t[:, mc, 0:npts], psums[mc][:, 0:npts],
                            float(BETA), bias, ALU.mult, ALU.add,
                        )
                    else:
                        nc.scalar.activation(
                            u_t[:, mc, 0:npts], psums[mc][:, 0:npts],
                            AF.Identity, bias=bias, scale=float(BETA),
                        )
                return u_t

            def emit_chain(t0, nt, s, l, u_t):
                if SOFTPLUS_MODE == "table":
                    h_cur[t0] = u_t  # ACT already wrote H
                    return
                """softplus tail: H = max(u, min(ln(1+exp(u)), 88.70)).

                exp(u>88.7) -> Inf and ln(Inf) -> Inf, but min(t, 88.70)
                caps that; for u > 17 ln(1+e^u) == u in fp32, so the max
                picks the exact u branch everywhere the cap engages.
                Full-tile ops; the unused half of a single-tile step just
                computes garbage that nothing reads."""
                e_t = e_pool.tile([P, MC, PAIR * PT], f32,
                                  name=f"e_{t0}_{l}", tag="e")
                t_t = e_pool.tile([P, MC, PAIR * PT], f32,
                                  name=f"t_{t0}_{l}", tag="e")
                nc.scalar.activation(e_t[:], u_t[:], AF.Exp)
                nc.scalar.activation(t_t[:], e_t[:], AF.Ln, bias=1.0)
                if mm_mode == "bf16":
                    h_t = hb_pool.tile([P, MC, PAIR * PT], wdt,
                                       name=f"h_{t0}_{l}", tag="hb")
                else:
                    h_t = u_t  # in-place: u becomes H
                nc.vector.scalar_tensor_tensor(
                    h_t[:], t_t[:], 88.70, u_t[:], ALU.min, ALU.max,
                )
                h_cur[t0] = h_t

            def emit_final(t0, szs, s):
                npts = sum(szs)
                locs = [sum(szs[:i]) for i in range(len(szs))]
                h_prev = h_cur[t0]
                ps8 = ps_pool.tile([1, PAIR * PT], f32, name=f"ps8_{t0}", tag="ps")
                if FP8 and L8_DVE:
                    # Output layer: fold wo into A7 on the mostly-idle
                    # DVE (g = sum_kc wo_kc*A7_kc, SBUF-only; Pool lacks
                    # TensorScalarPtr), so the PE pays one ones-contraction
                    # matmul per bank chunk instead of four wo-column
                    # matmuls (~1.3us/cell PE).
                    g_t = uh_pool.tile([P, PAIR * PT], wdt,
                                       name=f"g_{t0}", tag="uh")
                    nc.vector.tensor_scalar(
                        g_t[:, 0:npts], h_prev[:, 0, 0:npts],
                        wov_sb[s][:, 0:1], None, ALU.mult,
                    )
                    for kc in range(1, KC):
                        nc.vector.scalar_tensor_tensor(
                            g_t[:, 0:npts], h_prev[:, kc, 0:npts],
                            wov_sb[s][:, kc:kc + 1], g_t[:, 0:npts],
                            ALU.mult, ALU.add,
                        )
                    for loc, sz in zip(locs, szs):
                        nc.tensor.matmul(
                            ps8[0:1, loc:loc + sz], ones_t[:],
                            g_t[:, loc:loc + sz], start=True, stop=True,
                        )
                else:
                    for kc in range(KC):
                        for loc, sz in zip(locs, szs):
                            nc.tensor.matmul(
                                ps8[0:1, loc:loc + sz],
                                wo_sb[s][:, kc:kc + 1],
                                h_prev[:, kc, loc:loc + sz],
                                start=(kc == 0), stop=(kc == KC - 1),
                            )
                o_t = out_pool.tile([1, PAIR * PT], f32, name=f"o_{t0}", tag="o")
                nc.vector.tensor_scalar(
                    o_t[0:1, 0:npts], ps8[0:1, 0:npts],
                    bo_sb[s][0:1, 0:1], None, ALU.add,
                )
                nc.sync.dma_start(
                    out_d[0:1, t0:t0 + npts], o_t[0:1, 0:npts]
                )

            # Wavefront emission: cell (l, step) runs on diagonal 2l+step
            # (slope 2).  Mixing cheap-PE L0 cells with hidden-layer cells
            # keeps the PE utilization high while ACT digests the L0
            # evacuations — a layer-major order idles the PE there, which
            # both wastes the wait and trips the DVFS throttle to 1.2 GHz.
            # Slope 2 (vs 1) gives each cell TWO diagonals of slack on its
            # A(l-1) dependency and dilutes the PE-light L0 cells among
            # hidden-layer matmuls, which removes the once-per-diagonal
            # PE/ACT lockstep stalls seen in the fill phase.
            # l == N_HID+1 is the final (output-layer) cell of a step.
            for grp in groups:
                n = len(grp)
                for diag in range(2 * (N_HID + 2) + n - 1):
                    for si in range(n):
                        if (diag - si) % 2:
                            continue
                        l = (diag - si) // 2
                        if l < 0 or l > N_HID + 1:
                            continue
                        t0, szs, s = grp[si]
                        if l <= N_HID:
                            u_t = emit_mms(t0, szs, s, l)
                            emit_chain(t0, szs, s, l, u_t)
                        else:
                            emit_final(t0, szs, s)

    # Drop InstLdweights whose weights AP matches the immediately
    # preceding load on the PE stream (walrus --enable-ldw-opt does the
    # same dedup but its codegen path asserts on this program).  The PE
    # array keeps the stationary operand across matmuls, so a reload of
    # the identical AP is pure overhead (~46ns/matmul measured).  Dep
    # edges of the removed load are merged into the following matmul;
    # nothing references an InstLdweights by name (verified).
    def _dedup_ldweights():
        n_removed = 0
        for blk in nc.main_func.blocks:
            insts = blk.instructions
            last_sig = None
            pending = None
            to_remove = []
            for i in insts:
                if isinstance(i, mybir.InstLdweights):
                    s = (str(i.ins[0]), str(i.tile_position),
                         str(i.tile_size), str(i.perf_mode),
                         str(i.is_transpose))
                    if s == last_sig:
                        to_remove.append(i)
                        pending = i
                    else:
                        last_sig = s
                        pending = None
                elif isinstance(i, mybir.InstMatmult):
                    if pending is not None:
                        i.merge_dependencies_from(pending)
                        pending = None
            for i in to_remove:
                insts.remove(i)
            n_removed += len(to_remove)
        return n_removed

    _dedup_ldweights()

    # Pin Exp+Ln to the one table set containing both, so the ACT engine
    # doesn't reload tables between every exp and ln.
    import concourse.bacc as bacc_mod
    import concourse.hw_specs as hw_specs
    _real_tables = hw_specs.get_activation_tables
    _keep = "natural_log_exp_and_others"

    def _pinned_tables(arch):
        t = _real_tables(arch)
        return {
            name: (funcs if name == _keep else (funcs - {AF.Exp, AF.Ln}))
            for name, funcs in t.items()
        }

    bacc_mod.get_activation_tables = _pinned_tables
    try:
        nc.compile()
    finally:
        bacc_mod.get_activation_tables = _real_tables
    return nc


# --------------------------------------------------------------------------
# kernel()
# --------------------------------------------------------------------------

def _maybe_patch_ldw_opt():
    """Optionally flip walrus's --enable-ldw-opt (dedups back-to-back
    LDWEIGHTS of the same stationary operand). Gated by env for A/B."""
    import concourse.bass_utils as bu

    if _os.environ.get("KERNEL_LDW_OPT") != "1":
        return
    if getattr(bu.run_command, "_ldw_patched", False):
        return
    orig = bu.run_command

    def patched(argv, **kw):
        argv = [
            "--enable-ldw-opt=true" if a == "--enable-ldw-opt=false" else a
            for a in argv
        ]
        return orig(argv, **kw)

    patched._ldw_patched = True
    bu.run_command = patched


def kernel(x, type_vec, W0, b0, Wh, bh, Wo, bo):
    from concourse.bass_utils import run_bass_kernel_spmd
    import ml_dtypes

    _maybe_patch_ldw_opt()
    if SOFTPLUS_MODE == "table":
        _os.environ["BASS_ACT_ROOT_JSON_PATH"] = _gen_act_tables(
            HS if FP8 else 1.0)

    x = np.ascontiguousarray(np.asarray(x, dtype=np.float32))
    tv = np.asarray(type_vec).astype(np.int64)
    W0 = np.asarray(W0, dtype=np.float32)
    b0 = np.asarray(b0, dtype=np.float32)
    Wh = np.asarray(Wh, dtype=np.float32)
    bh = np.asarray(bh, dtype=np.float32)
    Wo = np.asarray(Wo, dtype=np.float32)
    bo = np.asarray(bo, dtype=np.float32)
    N = x.shape[0]

    counts = np.bincount(tv, minlength=T)
    starts = np.concatenate([[0], np.cumsum(counts)])
    shape, asg = _plan(counts)
    S = len(shape)

    # rebalance: when one expert is split over several cores' same-index
    # slots, equalize the piece sizes so the max (which sets the shared
    # cap) is minimal
    for e, takes in asg.items():
        by_slot = {}
        for i, (c, s, amt) in enumerate(takes):
            by_slot.setdefault(s, []).append(i)
        for s, idxs in by_slot.items():
            if len(idxs) < 2:
                continue
            tot = sum(takes[i][2] for i in idxs)
            base, rem = divmod(tot, len(idxs))
            for j, i in enumerate(idxs):
                c, s_, _old = takes[i]
                takes[i] = (c, s_, base + (1 if j < rem else 0))

    # shrink each slot to the max points any core actually uses, rounded to
    # a multiple of 4 (ragged last tile), to cut padding compute
    used = np.zeros(S, dtype=np.int64)
    for e, takes in asg.items():
        for (c, s, amt) in takes:
            used[s] = max(used[s], amt)
    caps = tuple(int(max(128, -(-u // 4) * 4)) for u in used)
    NP = sum(caps)
    phase_off = np.concatenate([[0], np.cumsum(np.asarray(caps))])

    # per-core slot -> expert, and gathered point indices
    slot_expert = np.zeros((N_CORES, S), dtype=np.int64)
    gidx = np.full((N_CORES, NP), -1, dtype=np.int64)
    for e, takes in asg.items():
        pos = int(starts[e])
        for (c, s, amt) in takes:
            o = int(phase_off[s])
            gidx[c, o:o + amt] = np.arange(pos, pos + amt)
            slot_expert[c, s] = e
            pos += amt

    np_wdt = ml_dtypes.bfloat16 if MM_MODE == "bf16" else np.float32
    np_hdt = ml_dtypes.float8_e4m3 if FP8 else np_wdt

    # pre-transposed / pre-scaled weight views per expert
    w0t_e = np.ascontiguousarray(W0.transpose(0, 2, 1))            # [T,67,H]
    # fp8: weights scaled into e4m3's normal range; activations carry
    # x HS (in the table); the ACT evac scale removes both.  bf16: fold
    # 1/BETA into the weights as before.
    whs = ((Wh * ALPHA) if FP8 else (Wh / BETA)).astype(np.float32)
    wht_e = np.ascontiguousarray(
        whs.transpose(0, 1, 3, 2).reshape(T, N_HID, KC, P, H).transpose(0, 1, 3, 2, 4)
    )                                                              # [T,7,P,KC,H]
    wot_e = np.ascontiguousarray(
        (Wo / (BETA * HS if FP8 else BETA))
        .reshape(T, H).reshape(T, KC, P).transpose(0, 2, 1)
    )                                                              # [T,P,KC]
    b0v_e = np.ascontiguousarray((BETA * b0).reshape(T, MC, P).transpose(0, 2, 1))
    bhv_e = np.ascontiguousarray(
        (BETA * bh).reshape(T, N_HID, MC, P).transpose(0, 3, 1, 2)
    )                                                              # [T,P,7,MC]
    bov_e = bo.reshape(T, 1)

    in_maps = []
    for c in range(N_CORES):
        sel = np.where(gidx[c] >= 0, gidx[c], 0)
        xg = x[sel]                                                # [NP, 67]
        ex = slot_expert[c]
        in_maps.append({
            "xT": np.ascontiguousarray(xg.T).astype(np_wdt),
            "w0t": w0t_e[ex].astype(np_wdt),
            "wht": wht_e[ex].astype(np_hdt),
            "wot": wot_e[ex].astype(np_wdt),
            "wov": np.ascontiguousarray(wot_e[ex], dtype=np.float32),
            "b0v": b0v_e[ex],
            "bhv": bhv_e[ex],
            "bov": bov_e[ex],
        })

    key = (caps, MM_MODE, SOFTPLUS_MODE, FP8)
    # the custom-DVE softplus must be registered while the program is
    # built AND while the BIR compiles (inside run_bass_kernel_spmd);
    # restore the registry before returning
    _register_sp_pw3()
    try:
        if key not in _nc_cache:
            _nc_cache[key] = _build_nc(caps, MM_MODE)
        nc = _nc_cache[key]

        res = run_bass_kernel_spmd(nc, in_maps, core_ids=list(range(N_CORES)))
    finally:
        _unregister_sp_pw3()
    global _last_results
    _last_results = res

    out = np.zeros((N, OUT), dtype=np.float32)
    for c in range(N_CORES):
        oc = res.results[c]["out"].reshape(-1)
        m = gidx[c] >= 0
        out[gidx[c][m], 0] = oc[m]
    return out



# revision 33
# speedup vs baseline: 1.1395x; 1.1395x over previous
"""EnsembleDeepSDF MoE-routing kernel for 8 Trainium2 NeuronCores.

Strategy: the harness calls kernel(**inputs) with the full inputs; we do all
routing on the host.  type_vec is sorted, so each expert owns a contiguous
segment of points.  We pick a per-core "phase shape" (tile counts per weight
slot, identical on every core so one SPMD program serves all 8 cores), pack
the 9 experts' segments into the 8*len(shape) single-expert slots, gather
each core's points (padding with point 0), and hand each core its own
pre-transposed/pre-scaled weight slots as inputs.  The device program is a
straight-line Tile kernel: per point-tile, 9 matmul layers with softplus
activations.

softplus: the compiler's ACT tables have no softplus, so we generate a
custom piecewise-cubic table (same binary format as the shipped sets,
reverse-engineered from exp's entries) that replaces `exp` with
softplus(x) = ln(1+e^x), and point the compiler at it via
BASS_ACT_ROOT_JSON_PATH.  One ACT op then does the whole activation
including the PSUM evacuation and the beta scale/bias fma (free on ACT).
A fallback "exact" mode (exp+ln+fused clamp/max on stock tables) is kept
behind KERNEL_SOFTPLUS=exact.

The torch Softplus(beta=100) is softplus(100*z)/100; we keep activations in
the H = softplus(100*z) domain and fold the 1/100 into the next layer's
weights host-side, so no extra scaling ops run on device.
"""

import json
import os as _os
import shutil
import tempfile

import numpy as np

T, D_IN, H, OUT, N_HID = 9, 67, 512, 1, 7
BETA = 100.0
N_CORES = 8
PT = 512          # points per tile (one PSUM bank of fp32)
P = 128           # partitions
KC = H // P       # 4 contraction chunks
MC = H // P       # 4 output-feature chunks
# point-tiles per pipeline step (psum tile = PAIR banks; PSUM holds
# 8//PAIR tiles).  PAIR=4 would halve the per-op ACT pipeline-fill
# overhead and amortize each DoubleRow LDWEIGHTS over 4 matmuls, but the
# 2-deep (vs 4) psum rotation couples PE and ACT into per-chunk lockstep
# and measures 13% SLOWER end to end; PAIR=2 is the sweet spot.
PAIR = int(_os.environ.get("KERNEL_PAIR", "2"))

# matmul dtype knob: "f32" (exact, 4 cyc/col), "f32r" (tf32-ish, 1 cyc/col),
# "bf16" (1 cyc/col, FWL halves LDWEIGHTS, halves DMA/SBUF bytes)
MM_MODE = _os.environ.get("KERNEL_MM_MODE", "bf16")
# "table" = custom softplus ACT table (1 op); "exact" = exp/ln chain
SOFTPLUS_MODE = _os.environ.get("KERNEL_SOFTPLUS", "table")
# fp8 DoubleRow for the 7 hidden-layer matmuls: weights are scaled into
# e4m3's normal range (x ALPHA) and activations carry a global x HS fold
# (baked into the softplus table); the ACT evacuation scale removes both.
# Measured numpy end-to-end rel err 4.6e-3 vs the 2e-2 gate.
FP8 = _os.environ.get("KERNEL_FP8", "1") != "0" and SOFTPLUS_MODE == "table" \
    and MM_MODE == "bf16"
HS = 0.125        # activation scale folded into the table
# fp8 weight scale.  ALPHA*HS == 1 so the hidden-layer PSUM arrives already
# in the BETA*z domain (ACT scale 1.0; the custom-DVE softplus needs no
# scale constant on its input).  max |Wh|*8 ~ 0.35, inside e4m3's normal
# range (min normal 2^-6).
ALPHA = 8.0
# custom-DVE piecewise softplus for half the hidden-layer evacuations.
# Numerically validated (rel err 5.7e-3) and compiles, but the runtime
# crashes executing the NEFF: the byte-36 row dispatch for custom-DVE ops
# appears to be baked into firmware (ant_dve_dispatch.hpp), so a row the
# firmware doesn't know is fatal.  Kept for documentation; off by default.
DVE_SP = _os.environ.get("KERNEL_DVE_SP", "0") != "0"
# fold wo into A7 on DVE + ones-matmul on PE for the output layer.
# Measured neutral (the stream is ACT-paced, so trimming PE's output-layer
# matmuls buys nothing) and it adds a bf16 rounding of the partial sums;
# kept behind a knob, off by default.
L8_DVE = _os.environ.get("KERNEL_L8_DVE", "0") != "0"

_nc_cache = {}
_last_results = None

# --------------------------------------------------------------------------
# Custom DVE op: 3-tangent piecewise-linear HS*softplus with bias add.
# out = max(0, HS/2*u + HS*(ln2+0.11), HS*u), u = psum + bias.  The +0.11
# lift centers the one-sided tangent error; measured end-to-end rel-err
# cost vs the exact table is ~3e-4.  Offloads 2 of 4 evacuation chunks per
# hidden layer from the saturated ACT engine (the softplus table runs only
# there) onto the mostly-idle DVE.
# --------------------------------------------------------------------------

_SP_PW3 = None
_SP_PW3_NAME = "SOFTPLUS_PW3_ANT"


def _get_sp_pw3():
    global _SP_PW3
    if _SP_PW3 is not None:
        return _SP_PW3
    from concourse.dve_spec import Spec, Src0, Src1, C0, C1, C2, Zero, maxx
    from concourse.dve_spec import _has_src1, lower
    from concourse.dve_uop import DveOpSpec
    import concourse.dve_ops as dops

    u = Src0 + Src1
    spec = Spec(body=maxx(maxx(C1 * u + C2, C0 * u), Zero))
    row = dops._CUSTOM_DVE_ROW_BASE + len(dops.OPS)
    shas = {}
    for ver in ("v3", "v4"):
        try:
            s = DveOpSpec(name=_SP_PW3_NAME, opcode=row,
                          uops=lower(spec, ver=ver), rd1_en=_has_src1(spec))
            shas[ver] = s.sha(ver)
        except Exception:
            pass
    _SP_PW3 = dops.DveOp(_SP_PW3_NAME, spec, subdim=False, uops_sha=shas)
    return _SP_PW3


def _register_sp_pw3():
    import concourse.dve_ops as dops
    op = _get_sp_pw3()
    if any(o.name == op.name for o in dops.OPS):
        return
    dops.OPS.append(op)
    dops.CUSTOM_DVE_SPECS[op.name] = op.spec
    dops._SUB_OPCODE_FOR_NAME[op.name] = (
        dops._CUSTOM_DVE_ROW_BASE + len(dops.OPS) - 1
    )


def _unregister_sp_pw3():
    import concourse.dve_ops as dops
    dops.OPS[:] = [o for o in dops.OPS if o.name != _SP_PW3_NAME]
    dops.CUSTOM_DVE_SPECS.pop(_SP_PW3_NAME, None)
    dops._SUB_OPCODE_FOR_NAME.pop(_SP_PW3_NAME, None)


# --------------------------------------------------------------------------
# Custom ACT table: replace `exp` with softplus in the shipped PWL sets.
# --------------------------------------------------------------------------

_ACT_SET = "natural_log_exp_and_others"
_act_table_dir = None


def _softplus64(x):
    x = np.asarray(x, dtype=np.float64)
    return np.log1p(np.exp(-np.abs(x))) + np.maximum(x, 0.0)


def _fit_cubic(a, b, hs):
    x0 = 0.5 * (a + b)
    k = np.arange(96)
    xs = x0 + 0.5 * (b - a) * np.cos(np.pi * (k + 0.5) / 96)
    c = np.polyfit(xs - x0, hs * _softplus64(xs), 3)
    return float(c[3]), float(c[2]), float(c[1]), float(c[0]), float(x0)


def _gen_act_tables(hs=1.0):
    """Build an act-root dir where `exp` computes hs*softplus. Returns the
    act_info.json path. The bucket entry layout ([d0,d1,d2,d3,x0,0,0,0],
    cubic in (x-x0)) and the per-exponent band structure are read from the
    shipped set so only coefficients and profile thresholds change."""
    global _act_table_dir
    if _act_table_dir is not None:
        return _act_table_dir
    from neuronxcc.driver.Job import Job
    from neuronxcc.driver.jobs.support.FindActInfo import findActInfoFile

    src_json = findActInfoFile(Job.getPackageDir(), "gen3")
    src = _os.path.dirname(src_json)
    out = _os.path.join(tempfile.mkdtemp(prefix="act_softplus_"), "tables")
    shutil.copytree(src, out)
    for f in _os.listdir(out):
        _os.chmod(_os.path.join(out, f), 0o644)

    d = json.load(open(f"{out}/{_ACT_SET}.json"))
    bkt = np.fromfile(f"{out}/{_ACT_SET}_bkt.bin", dtype=np.uint32)
    bkt = bkt.reshape(-1, 8).copy()
    fbkt = bkt.view(np.float32)
    e2b = {int(k): v for k, v in d["func_exp_to_bkt_start_idx"]["exp"].items()}
    prof = [p for p in d["profile_meta_data"] if p["func_name"] == "exp_400p"][0]

    def put(idx, d0, d1, d2, d3, x0):
        fbkt[idx, 0:5] = np.array([d0, d1, d2, d3, x0], dtype=np.float32)
        bkt[idx, 5:8] = 0

    nseg = {-1: 2, 0: 4, 1: 8, 2: 16, 3: 32}
    for e in range(-19, 4):
        n = nseg.get(e, 1)
        neg_base, pos_base = e2b[e]
        A = 2.0 ** e
        h = A / n
        for k in range(n):
            a, b = A + k * h, A + (k + 1) * h
            put(pos_base + k, *_fit_cubic(a, b, hs))
            put(neg_base + k, *_fit_cubic(-b, -a, hs))

    ln2 = float(np.log(2.0))
    put(prof["pos_small_signal_pwl_control"],
        hs * ln2, hs * 0.5, hs * 0.125, 0.0, 0.0)
    put(prof["neg_small_signal_pwl_control"],
        hs * ln2, hs * 0.5, hs * 0.125, 0.0, 0.0)
    put(prof["pos_large_signal_pwl_control"], 0.0, hs, 0.0, 0.0, 0.0)
    put(prof["neg_large_signal_pwl_control"], 0.0, 0.0, 0.0, 0.0, 0.0)
    prof["large_pos_signal_exp_threshold"] = 131   # |x| >= 16 -> linear/zero
    prof["large_pos_signal_mantissa_threshold"] = 0
    prof["large_neg_signal_exp_threshold"] = 131
    prof["large_neg_signal_mantissa_threshold"] = 0
    prof["fzero_result"] = int(np.float32(hs * ln2).view(np.uint32))
    prof["fninf_result"] = 0
    prof["fpinf_result"] = 2139095040

    bkt.tofile(f"{out}/{_ACT_SET}_bkt.bin")
    with open(f"{out}/{_ACT_SET}.json", "w") as f:
        json.dump(d, f)
    _act_table_dir = _os.path.join(out, "act_info.json")
    return _act_table_dir


# --------------------------------------------------------------------------
# Host-side planning: pack expert segments into 8 x len(shape) slots.
# --------------------------------------------------------------------------

def _try_pack(shape, counts):
    """Assign experts to single-expert slots. Slot (c, s) holds shape[s]*PT
    points. Returns {expert: [(core, s, amount), ...]} or None."""
    slots = []  # (capacity, core, s)
    for s, t in enumerate(shape):
        for c in range(N_CORES):
            slots.append([t * PT, c, s])
    experts = sorted(
        [e for e in range(T) if counts[e] > 0], key=lambda e: -counts[e]
    )
    asg = {}
    avail = sorted(slots)  # by capacity asc
    for e in experts:
        need = int(counts[e])
        # smallest single slot that fits
        one = next((sl for sl in avail if sl[0] >= need), None)
        if one is not None:
            asg[e] = [(one[1], one[2], need)]
            avail.remove(one)
            continue
        # greedily take largest slots
        take = []
        rem = need
        pool = sorted(avail, key=lambda sl: -sl[0])
        for sl in pool:
            if rem <= 0:
                break
            amt = min(rem, sl[0])
            take.append((sl[1], sl[2], amt))
            rem -= amt
            avail.remove(sl)
        if rem > 0:
            return None
        asg[e] = take
    return asg


def _plan(counts):
    cands = set()
    for t1 in range(1, 17):
        cands.add((t1,))
        for t2 in range(1, t1 + 1):
            cands.add((t1, t2))
            for t3 in range(1, t2 + 1):
                cands.add((t1, t2, t3))
    for shape in sorted(cands, key=lambda s: (sum(s), len(s))):
        asg = _try_pack(shape, counts)
        if asg is not None:
            return shape, asg
    raise RuntimeError("no feasible slot shape")


# --------------------------------------------------------------------------
# Device program
# --------------------------------------------------------------------------

def _build_nc(caps, mm_mode):
    import concourse.bass as bass
    import concourse.tile as tile
    import concourse.mybir as mybir
    from concourse import bacc

    f32 = mybir.dt.float32
    AF = mybir.ActivationFunctionType
    ALU = mybir.AluOpType
    if mm_mode == "bf16":
        wdt = mybir.dt.bfloat16   # weights/x/h (matmul operands)
        udt = f32                 # u stays f32; h is a separate bf16 tile
    elif mm_mode == "f32r":
        wdt = mybir.dt.float32r
        udt = mybir.dt.float32r   # u doubles as h (in-place max)
    else:
        wdt = f32
        udt = f32

    S = len(caps)
    NP = sum(caps)

    hdt = mybir.dt.float8e4 if FP8 else wdt   # hidden-layer matmul operands

    nc = bacc.Bacc("TRN2", target_bir_lowering=False)
    xT_in = nc.dram_tensor("xT", [D_IN, NP], wdt, kind="ExternalInput")
    w0t_in = nc.dram_tensor("w0t", [S, D_IN, H], wdt, kind="ExternalInput")
    wht_in = nc.dram_tensor("wht", [S, N_HID, P, KC, H], hdt, kind="ExternalInput")
    wot_in = nc.dram_tensor("wot", [S, P, KC], wdt, kind="ExternalInput")
    wov_in = nc.dram_tensor("wov", [S, P, KC], f32, kind="ExternalInput")
    b0v_in = nc.dram_tensor("b0v", [S, P, MC], f32, kind="ExternalInput")
    bhv_in = nc.dram_tensor("bhv", [S, P, N_HID, MC], f32, kind="ExternalInput")
    bov_in = nc.dram_tensor("bov", [S, 1], f32, kind="ExternalInput")
    out_d = nc.dram_tensor("out", [1, NP], f32, kind="ExternalOutput")

    # steps: (point_offset, (tile_sizes...), slot); each step's tiles go in
    # one PSUM tile (first tile bank-aligned at 512, total <= 1024)
    steps = []
    off = 0
    for s, cap in enumerate(caps):
        sizes = [PT] * (cap // PT)
        if cap % PT:
            sizes.append(cap % PT)
        i = 0
        while i < len(sizes):
            take = sizes[i:i + PAIR]
            steps.append((off, tuple(take), s))
            off += sum(take)
            i += PAIR

    NSTREAM = int(_os.environ.get("KERNEL_NSTREAM", "0")) or len(steps)
    NWARM = int(_os.environ.get("KERNEL_NWARM", "112"))

    with tile.TileContext(nc) as tc:
        with (
            tc.tile_pool(name="xin", bufs=len(steps)) as xin_pool,
            tc.tile_pool(name="wts", bufs=1) as wts_pool,
            tc.tile_pool(
                name="whp",
                bufs=(S * N_HID if mm_mode == "bf16"
                      else min(10 if SOFTPLUS_MODE == "table" else 7, S * N_HID)),
            ) as wh_pool,
            tc.tile_pool(name="uh", bufs=3 if mm_mode == "bf16" else 2 * NSTREAM) as uh_pool,
            tc.tile_pool(name="hb", bufs=2 * NSTREAM) as hb_pool,
            tc.tile_pool(name="ebuf", bufs=2) as e_pool,
            tc.tile_pool(name="outp", bufs=2) as out_pool,
            tc.tile_pool(name="ps", bufs=8 // PAIR, space="PSUM") as ps_pool,
        ):
            groups = [steps[i:i + NSTREAM] for i in range(0, len(steps), NSTREAM)]
            xT_sb = {}
            h_cur = {}

            # DMA trigger economics: each dma_start costs ~1.3us of the
            # issuing engine's sequencer, triggers on one queue serialize,
            # and one trigger's transfer runs on a single DMA engine
            # (~22.5 GB/s).  So the first-layer-critical loads are split
            # across BOTH hwdge queues (sync + scalar; scalar is free until
            # the first softplus at ~6us) with 2-way transfer splits, and
            # the long weight tail streams on the gpsimd software-DGE queue
            # in consumption order.
            w0_sb, wo_sb, b0_sb, bh_sb, bo_sb = [None] * S, [None] * S, [None] * S, [None] * S, [None] * S
            wov_sb = [None] * S
            wh_sb = [[None] * N_HID for _ in range(S)]

            def load_wh(s, l, splits):
                """splits: list of (engine, kc_lo, kc_hi) DMA pieces.  Layer
                matmuls consume kc chunks in order, and Tile's AP-range deps
                let kc0/1 matmuls start before kc2/3 land."""
                wh_t = wh_pool.tile([P, KC, H], hdt, name=f"wh_{s}_{l}", tag="wh")
                for eng, lo, hi in splits:
                    eng.dma_start(wh_t[:, lo:hi, :], wht_in[s, l, :, lo:hi, :])
                wh_sb[s][l] = wh_t

            def load_x(t0, szs, eng, nsplit=1):
                x_t = xin_pool.tile([D_IN, PAIR * PT], wdt,
                                    name=f"x_{t0}", tag="x")
                w = sum(szs)
                step = (w + nsplit - 1) // nsplit
                for o in range(0, w, step):
                    e = min(o + step, w)
                    eng.dma_start(x_t[:, o:e], xT_in[:, t0 + o:t0 + e])
                xT_sb[t0] = x_t

            def load_w0b0(s, eng):
                w0_t = wts_pool.tile([D_IN, H], wdt, name=f"w0_{s}")
                eng.dma_start(w0_t[:], w0t_in[s])
                w0_sb[s] = w0_t
                b0_t = wts_pool.tile([P, MC], f32, name=f"b0_{s}")
                eng.dma_start(b0_t[:], b0v_in[s])
                b0_sb[s] = b0_t

            def load_bh(s, eng):
                bh_t = wts_pool.tile([P, N_HID, MC], f32, name=f"bh_{s}")
                eng.dma_start(bh_t[:], bhv_in[s])
                bh_sb[s] = bh_t

            def load_wobo(s, eng):
                wo_t = wts_pool.tile([P, KC], wdt, name=f"wo_{s}")
                eng.dma_start(wo_t[:], wot_in[s])
                wo_sb[s] = wo_t
                wov_t = wts_pool.tile([P, KC], f32, name=f"wov_{s}")
                eng.dma_start(wov_t[:], wov_in[s])
                wov_sb[s] = wov_t
                bo_t = wts_pool.tile([1, 1], f32, name=f"bo_{s}")
                eng.dma_start(bo_t[:], bov_in[s:s + 1, 0:1])
                bo_sb[s] = bo_t

            # pre-warm the ACT table set during the initial DMA wait: a
            # dependency-free dummy op carries the one-time table load
            warm_t = wts_pool.tile([1, 1], f32, name="warm")
            nc.vector.memset(warm_t[:], 0.0)
            nc.scalar.activation(warm_t[:], warm_t[:], AF.Exp)
            # ones column for the output layer's cross-partition sum
            ones_t = wts_pool.tile([P, 1], wdt, name="ones")
            nc.vector.memset(ones_t[:], 1.0)

            # Small weight/bias tensors ride the sync HWDGE queue — its own
            # sequencer issues triggers (~1.2us each) in parallel with the
            # gpsimd queue's, so the first-layer critical set (w0+b0) lands
            # ~10us earlier than when queued behind the x pieces.  x and
            # the wh bulk stay on the gpsimd SWDGE queue (8 DMA engines, 8
            # transfers in flight, in-order completion ring), in
            # consumption order: per layer, alternating slots.
            slot_order = []
            for (_t0, _szs, s) in steps:
                if s not in slot_order:
                    slot_order.append(s)
            s0 = slot_order[0]

            for s in slot_order:
                load_w0b0(s, nc.sync)
                load_bh(s, nc.sync)

            for (t0, szs, _s) in steps[:2]:
                load_x(t0, szs, nc.gpsimd, nsplit=2)
            load_wh(s0, 0, [(nc.gpsimd, 0, 1), (nc.gpsimd, 1, 2),
                            (nc.gpsimd, 2, 3), (nc.gpsimd, 3, 4)])
            for (t0, szs, _s) in steps[2:4]:
                load_x(t0, szs, nc.gpsimd, nsplit=2)
            for (t0, szs, _s) in steps[4:]:
                load_x(t0, szs, nc.gpsimd)
            for s in slot_order:
                load_wobo(s, nc.sync)
            for l in range(N_HID):
                for s in slot_order:
                    if s == s0 and l == 0:
                        continue
                    load_wh(s, l, [(nc.gpsimd, 0, 2), (nc.gpsimd, 2, 4)])

            # HAM warm-up: dependency-free matmuls on a memset tile keep the
            # PE busy (and the clock gate at 8/8 = 2.4 GHz) through the
            # initial x/weight DMA wait.  An idle PE not only wastes that
            # window — it re-throttles to 1.2 GHz and takes >3.4us of busy
            # time to recover, so bridging the whole wait is worth it.
            wdum = wts_pool.tile([P, P], wdt, name="wdum")
            nc.vector.memset(wdum[:], 0.0)
            ps_warm = ps_pool.tile([P, PAIR * PT], f32, name="ps_warm", tag="ps")
            for _ in range(NWARM):
                nc.tensor.matmul(
                    ps_warm[:, 0:P], wdum[:], wdum[:], start=True, stop=True
                )

            def emit_mms(t0, szs, s, l):
                """Matmuls for layer l + PSUM evacuation into u (the evac ops
                are emitted here so they sit at the head of the DVE/ACT queues
                and free PSUM slots promptly)."""
                npts = sum(szs)
                # tile-local offsets; all but the last tile are 512 so
                # every tile stays bank-aligned in PSUM
                locs = [sum(szs[:i]) for i in range(len(szs))]
                h_prev = h_cur.get(t0)
                psums = []
                for mc in range(MC):
                    ps_t = ps_pool.tile(
                        [P, PAIR * PT], f32, name=f"ps_{t0}_{l}_{mc}", tag="ps"
                    )
                    psums.append(ps_t)
                    if l == 0:
                        for loc, sz in zip(locs, szs):
                            nc.tensor.matmul(
                                ps_t[:, loc:loc + sz],
                                w0_sb[s][:, mc * P:(mc + 1) * P],
                                xT_sb[t0][:, loc:loc + sz],
                                start=True, stop=True,
                            )
                    elif FP8:
                        # fp8 DoubleRow: each matmul contracts a kc PAIR
                        # (2x128 rows, 2 fp8 weights per PE cell)
                        for kp in range(KC // 2):
                            for loc, sz in zip(locs, szs):
                                nc.tensor.matmul(
                                    ps_t[:, loc:loc + sz],
                                    wh_sb[s][l - 1][
                                        :, 2 * kp:2 * kp + 2,
                                        mc * P:(mc + 1) * P],
                                    h_prev[:, 2 * kp:2 * kp + 2,
                                           loc:loc + sz],
                                    start=(kp == 0), stop=(kp == KC // 2 - 1),
                                    perf_mode=mybir.MatmulPerfMode.DoubleRow,
                                )
                    else:
                        for kc in range(KC):
                            for loc, sz in zip(locs, szs):
                                nc.tensor.matmul(
                                    ps_t[:, loc:loc + sz],
                                    wh_sb[s][l - 1][:, kc, mc * P:(mc + 1) * P],
                                    h_prev[:, kc, loc:loc + sz],
                                    start=(kc == 0), stop=(kc == KC - 1),
                                )
                # activation tiles feeding fp8 matmuls are fp8 themselves
                adt = (mybir.dt.float8e4 if FP8 and l < N_HID else wdt)
                if SOFTPLUS_MODE == "table" and mm_mode == "bf16":
                    u_t = hb_pool.tile([P, MC, PAIR * PT], adt,
                                       name=f"u_{t0}_{l}", tag="hb")
                else:
                    u_t = uh_pool.tile([P, MC, PAIR * PT], udt,
                                       name=f"u_{t0}_{l}", tag="uh")
                # table arg must be BETA*z.  fp8 layers: psum = A@W_devT =
                # (HS*BETA*h)@(ALPHA*Wh)T -> scale 1/(HS*ALPHA); l==0: psum =
                # x@W0T = z0-b0 -> scale BETA; bf16 hidden (non-fp8): weights
                # pre-scaled 1/BETA host-side -> scale BETA.
                act_scale = float(BETA) if l == 0 or not FP8 \
                    else float(1.0 / (HS * ALPHA))
                for mc in range(MC):
                    bias = (b0_sb[s][:, mc:mc + 1] if l == 0
                            else bh_sb[s][:, l - 1, mc:mc + 1])
                    if (SOFTPLUS_MODE == "table" and FP8 and mc < 2
                            and 1 <= l < N_HID and DVE_SP):
                        # ACT is the saturated engine (softplus table); move
                        # half of each fp8 hidden layer's evacuation to DVE
                        # with the piecewise-linear softplus.  Needs the
                        # pre-scaled PSUM (ALPHA*HS == 1), so l == 0 (and
                        # the exact A7 for the cancellation-sensitive output
                        # layer) stay on the ACT table.
                        nc.vector._custom_dve(
                            _get_sp_pw3(),
                            out=u_t[:, mc, 0:npts],
                            in0=psums[mc][:, 0:npts],
                            in1=bias,
                            s0=float(HS), s1=float(HS / 2),
                            imm2=float(HS * (np.log(2.0) + 0.11)),
                        )
                    elif SOFTPLUS_MODE == "table":
                        # hijacked Exp == softplus; one ACT op does the
                        # evacuation + beta fma + activation
                        nc.scalar.activation(
                            u_t[:, mc, 0:npts], psums[mc][:, 0:npts],
                            AF.Exp, bias=bias, scale=act_scale,
                        )
                    elif mc < 3:
                        # u = 100*y + 100*b; evac split DVE (mc 0-2) / ACT (3)
                        nc.vector.tensor_scalar(
                            u_t[:, mc, 0:npts], psums[mc][:, 0:npts],
                            float(BETA), bias, ALU.mult, ALU.add,
                        )
                    else:
                        nc.scalar.activation(
                            u_t[:, mc, 0:npts], psums[mc][:, 0:npts],
                            AF.Identity, bias=bias, scale=float(BETA),
                        )
                return u_t

            def emit_chain(t0, nt, s, l, u_t):
                if SOFTPLUS_MODE == "table":
                    h_cur[t0] = u_t  # ACT already wrote H
                    return
                """softplus tail: H = max(u, min(ln(1+exp(u)), 88.70)).

                exp(u>88.7) -> Inf and ln(Inf) -> Inf, but min(t, 88.70)
                caps that; for u > 17 ln(1+e^u) == u in fp32, so the max
                picks the exact u branch everywhere the cap engages.
                Full-tile ops; the unused half of a single-tile step just
                computes garbage that nothing reads."""
                e_t = e_pool.tile([P, MC, PAIR * PT], f32,
                                  name=f"e_{t0}_{l}", tag="e")
                t_t = e_pool.tile([P, MC, PAIR * PT], f32,
                                  name=f"t_{t0}_{l}", tag="e")
                nc.scalar.activation(e_t[:], u_t[:], AF.Exp)
                nc.scalar.activation(t_t[:], e_t[:], AF.Ln, bias=1.0)
                if mm_mode == "bf16":
                    h_t = hb_pool.tile([P, MC, PAIR * PT], wdt,
                                       name=f"h_{t0}_{l}", tag="hb")
                else:
                    h_t = u_t  # in-place: u becomes H
                nc.vector.scalar_tensor_tensor(
                    h_t[:], t_t[:], 88.70, u_t[:], ALU.min, ALU.max,
                )
                h_cur[t0] = h_t

            def emit_final(t0, szs, s):
                npts = sum(szs)
                locs = [sum(szs[:i]) for i in range(len(szs))]
                h_prev = h_cur[t0]
                ps8 = ps_pool.tile([1, PAIR * PT], f32, name=f"ps8_{t0}", tag="ps")
                if FP8 and L8_DVE:
                    # Output layer: fold wo into A7 on the mostly-idle
                    # DVE (g = sum_kc wo_kc*A7_kc, SBUF-only; Pool lacks
                    # TensorScalarPtr), so the PE pays one ones-contraction
                    # matmul per bank chunk instead of four wo-column
                    # matmuls (~1.3us/cell PE).
                    g_t = uh_pool.tile([P, PAIR * PT], wdt,
                                       name=f"g_{t0}", tag="uh")
                    nc.vector.tensor_scalar(
                        g_t[:, 0:npts], h_prev[:, 0, 0:npts],
                        wov_sb[s][:, 0:1], None, ALU.mult,
                    )
                    for kc in range(1, KC):
                        nc.vector.scalar_tensor_tensor(
                            g_t[:, 0:npts], h_prev[:, kc, 0:npts],
                            wov_sb[s][:, kc:kc + 1], g_t[:, 0:npts],
                            ALU.mult, ALU.add,
                        )
                    for loc, sz in zip(locs, szs):
                        nc.tensor.matmul(
                            ps8[0:1, loc:loc + sz], ones_t[:],
                            g_t[:, loc:loc + sz], start=True, stop=True,
                        )
                else:
                    for kc in range(KC):
                        for loc, sz in zip(locs, szs):
                            nc.tensor.matmul(
                                ps8[0:1, loc:loc + sz],
                                wo_sb[s][:, kc:kc + 1],
                                h_prev[:, kc, loc:loc + sz],
                                start=(kc == 0), stop=(kc == KC - 1),
                            )
                o_t = out_pool.tile([1, PAIR * PT], f32, name=f"o_{t0}", tag="o")
                nc.vector.tensor_scalar(
                    o_t[0:1, 0:npts], ps8[0:1, 0:npts],
                    bo_sb[s][0:1, 0:1], None, ALU.add,
                )
                nc.sync.dma_start(
                    out_d[0:1, t0:t0 + npts], o_t[0:1, 0:npts]
                )

            # Wavefront emission: cell (l, step) runs on diagonal 2l+step
            # (slope 2).  Mixing cheap-PE L0 cells with hidden-layer cells
            # keeps the PE utilization high while ACT digests the L0
            # evacuations — a layer-major order idles the PE there, which
            # both wastes the wait and trips the DVFS throttle to 1.2 GHz.
            # Slope 2 (vs 1) gives each cell TWO diagonals of slack on its
            # A(l-1) dependency and dilutes the PE-light L0 cells among
            # hidden-layer matmuls, which removes the once-per-diagonal
            # PE/ACT lockstep stalls seen in the fill phase.
            # l == N_HID+1 is the final (output-layer) cell of a step.
            for grp in groups:
                n = len(grp)
                for diag in range(2 * (N_HID + 2) + n - 1):
                    for si in range(n):
                        if (diag - si) % 2:
                            continue
                        l = (diag - si) // 2
                        if l < 0 or l > N_HID + 1:
                            continue
                        t0, szs, s = grp[si]
                        if l <= N_HID:
                            u_t = emit_mms(t0, szs, s, l)
                            emit_chain(t0, szs, s, l, u_t)
                        else:
                            emit_final(t0, szs, s)

    # Drop InstLdweights whose weights AP matches the immediately
    # preceding load on the PE stream (walrus --enable-ldw-opt does the
    # same dedup but its codegen path asserts on this program).  The PE
    # array keeps the stationary operand across matmuls, so a reload of
    # the identical AP is pure overhead (~46ns/matmul measured).  Dep
    # edges of the removed load are merged into the following matmul;
    # nothing references an InstLdweights by name (verified).
    def _dedup_ldweights():
        n_removed = 0
        for blk in nc.main_func.blocks:
            insts = blk.instructions
            last_sig = None
            pending = None
            to_remove = []
            for i in insts:
                if isinstance(i, mybir.InstLdweights):
                    s = (str(i.ins[0]), str(i.tile_position),
                         str(i.tile_size), str(i.perf_mode),
                         str(i.is_transpose))
                    if s == last_sig:
                        to_remove.append(i)
                        pending = i
                    else:
                        last_sig = s
                        pending = None
                elif isinstance(i, mybir.InstMatmult):
                    if pending is not None:
                        i.merge_dependencies_from(pending)
                        pending = None
            for i in to_remove:
                insts.remove(i)
            n_removed += len(to_remove)
        return n_removed

    _dedup_ldweights()

    # Pin Exp+Ln to the one table set containing both, so the ACT engine
    # doesn't reload tables between every exp and ln.
    import concourse.bacc as bacc_mod
    import concourse.hw_specs as hw_specs
    _real_tables = hw_specs.get_activation_tables
    _keep = "natural_log_exp_and_others"

    def _pinned_tables(arch):
        t = _real_tables(arch)
        return {
            name: (funcs if name == _keep else (funcs - {AF.Exp, AF.Ln}))
            for name, funcs in t.items()
        }

    bacc_mod.get_activation_tables = _pinned_tables
    try:
        nc.compile()
    finally:
        bacc_mod.get_activation_tables = _real_tables
    return nc


# --------------------------------------------------------------------------
# kernel()
# --------------------------------------------------------------------------

def _maybe_patch_ldw_opt():
    """Optionally flip walrus's --enable-ldw-opt (dedups back-to-back
    LDWEIGHTS of the same stationary operand). Gated by env for A/B."""
    import concourse.bass_utils as bu

    if _os.environ.get("KERNEL_LDW_OPT") != "1":
        return
    if getattr(bu.run_command, "_ldw_patched", False):
        return
    orig = bu.run_command

    def patched(argv, **kw):
        argv = [
            "--enable-ldw-opt=true" if a == "--enable-ldw-opt=false" else a
            for a in argv
        ]
        return orig(argv, **kw)

    patched._ldw_patched = True
    bu.run_command = patched


def kernel(x, type_vec, W0, b0, Wh, bh, Wo, bo):
    from concourse.bass_utils import run_bass_kernel_spmd
    import ml_dtypes

    _maybe_patch_ldw_opt()
    if SOFTPLUS_MODE == "table":
        _os.environ["BASS_ACT_ROOT_JSON_PATH"] = _gen_act_tables(
            HS if FP8 else 1.0)

    x = np.ascontiguousarray(np.asarray(x, dtype=np.float32))
    tv = np.asarray(type_vec).astype(np.int64)
    W0 = np.asarray(W0, dtype=np.float32)
    b0 = np.asarray(b0, dtype=np.float32)
    Wh = np.asarray(Wh, dtype=np.float32)
    bh = np.asarray(bh, dtype=np.float32)
    Wo = np.asarray(Wo, dtype=np.float32)
    bo = np.asarray(bo, dtype=np.float32)
    N = x.shape[0]

    counts = np.bincount(tv, minlength=T)
    starts = np.concatenate([[0], np.cumsum(counts)])
    shape, asg = _plan(counts)
    S = len(shape)

    # rebalance: when one expert is split over several cores' same-index
    # slots, equalize the piece sizes so the max (which sets the shared
    # cap) is minimal
    for e, takes in asg.items():
        by_slot = {}
        for i, (c, s, amt) in enumerate(takes):
            by_slot.setdefault(s, []).append(i)
        for s, idxs in by_slot.items():
            if len(idxs) < 2:
                continue
            tot = sum(takes[i][2] for i in idxs)
            base, rem = divmod(tot, len(idxs))
            for j, i in enumerate(idxs):
                c, s_, _old = takes[i]
                takes[i] = (c, s_, base + (1 if j < rem else 0))

    # shrink each slot to the max points any core actually uses, rounded to
    # a multiple of 4 (ragged last tile), to cut padding compute
    used = np.zeros(S, dtype=np.int64)
    for e, takes in asg.items():
        for (c, s, amt) in takes:
            used[s] = max(used[s], amt)
    caps = tuple(int(max(128, -(-u // 4) * 4)) for u in used)
    NP = sum(caps)
    phase_off = np.concatenate([[0], np.cumsum(np.asarray(caps))])

    # per-core slot -> expert, and gathered point indices
    slot_expert = np.zeros((N_CORES, S), dtype=np.int64)
    gidx = np.full((N_CORES, NP), -1, dtype=np.int64)
    for e, takes in asg.items():
        pos = int(starts[e])
        for (c, s, amt) in takes:
            o = int(phase_off[s])
            gidx[c, o:o + amt] = np.arange(pos, pos + amt)
            slot_expert[c, s] = e
            pos += amt

    np_wdt = ml_dtypes.bfloat16 if MM_MODE == "bf16" else np.float32
    np_hdt = ml_dtypes.float8_e4m3 if FP8 else np_wdt

    # pre-transposed / pre-scaled weight views per expert
    w0t_e = np.ascontiguousarray(W0.transpose(0, 2, 1))            # [T,67,H]
    # fp8: weights scaled into e4m3's normal range; activations carry
    # x HS (in the table); the ACT evac scale removes both.  bf16: fold
    # 1/BETA into the weights as before.
    whs = ((Wh * ALPHA) if FP8 else (Wh / BETA)).astype(np.float32)
    wht_e = np.ascontiguousarray(
        whs.transpose(0, 1, 3, 2).reshape(T, N_HID, KC, P, H).transpose(0, 1, 3, 2, 4)
    )                                                              # [T,7,P,KC,H]
    wot_e = np.ascontiguousarray(
        (Wo / (BETA * HS if FP8 else BETA))
        .reshape(T, H).reshape(T, KC, P).transpose(0, 2, 1)
    )                                                              # [T,P,KC]
    b0v_e = np.ascontiguousarray((BETA * b0).reshape(T, MC, P).transpose(0, 2, 1))
    bhv_e = np.ascontiguousarray(
        (BETA * bh).reshape(T, N_HID, MC, P).transpose(0, 3, 1, 2)
    )                                                              # [T,P,7,MC]
    bov_e = bo.reshape(T, 1)

    in_maps = []
    for c in range(N_CORES):
        sel = np.where(gidx[c] >= 0, gidx[c], 0)
        xg = x[sel]                                                # [NP, 67]
        ex = slot_expert[c]
        in_maps.append({
            "xT": np.ascontiguousarray(xg.T).astype(np_wdt),
            "w0t": w0t_e[ex].astype(np_wdt),
            "wht": wht_e[ex].astype(np_hdt),
            "wot": wot_e[ex].astype(np_wdt),
            "wov": np.ascontiguousarray(wot_e[ex], dtype=np.float32),
            "b0v": b0v_e[ex],
            "bhv": bhv_e[ex],
            "bov": bov_e[ex],
        })

    key = (caps, MM_MODE, SOFTPLUS_MODE, FP8)
    # the custom-DVE softplus must be registered while the program is
    # built AND while the BIR compiles (inside run_bass_kernel_spmd);
    # restore the registry before returning
    _register_sp_pw3()
    try:
        if key not in _nc_cache:
            _nc_cache[key] = _build_nc(caps, MM_MODE)
        nc = _nc_cache[key]

        res = run_bass_kernel_spmd(nc, in_maps, core_ids=list(range(N_CORES)))
    finally:
        _unregister_sp_pw3()
    global _last_results
    _last_results = res

    out = np.zeros((N, OUT), dtype=np.float32)
    for c in range(N_CORES):
        oc = res.results[c]["out"].reshape(-1)
        m = gidx[c] >= 0
        out[gidx[c][m], 0] = oc[m]
    return out

